# revision 7
# baseline (speedup 1.0000x reference)
"""GIN message-passing kernel for Trainium2, 8-core SPMD.

Strategy (graph/data parallel, edges partitioned by destination):
 - Core m owns destination nodes [12500*m, 12500*(m+1)).
 - Edges sorted by (dst supertile, src chunk, dst tile) and padded to
   128-edge blocks with block counts equalized across cores so all 8
   cores run one identical program (SPMD); per-core variation is data.
 - fp16 data path: node tables, gathered rows, one-hots, MLP weights
   and activations are fp16 (PSUM accumulation stays fp32), which runs
   the PE at 1 cycle/row (vs 4 for fp32) and halves DVE and collective
   traffic.
 - Per layer: dma_gather source rows from the full node table (chunks
   of <=32768 rows for int16 indices), segment-sum via PSUM-accumulated
   matmuls  agg[feat, dst] += G_block.T @ onehot(dst_local). One-hots
   for all blocks of a gather call are built with a single DVE is_equal
   over [128, gcnt*128] using a 3-D broadcast of the dst-local row.
 - MLP: feature-major matmuls with the 128x128 weights stationary; BN
   (eval) folded into per-partition scale/bias of a Relu activation.
 - Node-major h for the next layer's gather written via PE transpose;
   shards exchanged with quarter AllGathers that overlap compute. The
   input x is staged the same way on device (host ships only each
   core's 12500-row shard, not the replicated full table).
 - Pooling: per-graph-slot gathers (pad indices duplicate a real row of
   the same slot, neutral under max), PE transpose, reduce_max. The
   tiny [64,128] @ [128,4] classifier + log_softmax run on host.
"""

import hashlib
import os
import sys

sys.path.insert(0, "/opt/trn_rl_repo")

KM_DEBUG = os.environ.get("KM_DEBUG", "0") == "1"

from contextlib import ExitStack

import numpy as np

from concourse import bacc, mybir, tile
from concourse.masks import make_identity

F32 = mybir.dt.float32
F16 = mybir.dt.float16
I16 = mybir.dt.int16

N = 100_000
E = 3_200_000
D = 128
L = 3
K = 2
G = 64
C = 4
BN_EPS = 1e-5

NCORES = 8
SHARD = N // NCORES  # 12500
NTILES = 100  # 128-node dst tiles per core (12800 padded shard)
NSUP = 25  # supertiles of 512 dst nodes
NCHUNKS = 4  # src chunks == AllGather quarters (each <=28672 rows, int16-safe)
PAD_DST = 200.0  # local-dst sentinel for padded edges -> all-zero one-hot
NQUEUES = 4
GMAX = 24  # max 128-edge blocks per dma_gather call

# Shard quarters (supertile-aligned) for split AllGathers that overlap the
# tail of each layer's compute. The node table is stored PERMUTED:
# row(g) = QGOFF[q] + QSIZE[q]*m + (loc - QSTART[q]), where m = g//SHARD,
# loc = g%SHARD, q = quarter of loc. Each quarter's AllGather output is the
# rank-concatenation of that quarter across cores.
QSUP_LAST = [6, 12, 18, 24]  # last supertile of each quarter
QSTART = np.array([0, 3584, 6656, 9728])
QSIZE = np.array([3584, 3072, 3072, 2772])
QGOFF = np.array([0, 28672, 53248, 77824])


def _perm_rows(g):
    g = np.asarray(g, dtype=np.int64)
    m = g // SHARD
    loc = g % SHARD
    q = np.searchsorted(QSTART, loc, side="right") - 1
    return QGOFF[q] + QSIZE[q] * m + (loc - QSTART[q])

_compiled = {}
_runners = {}
_layout_cache = {}


def _build_edge_layout(edge_index):
    src = np.asarray(edge_index[0], dtype=np.int64)
    dst = np.asarray(edge_index[1], dtype=np.int64)

    core = dst // SHARD
    dloc = dst - core * SHARD
    t = dloc // 128  # local dst tile, 0..97
    s = t // 4  # supertile
    trel = t - s * 4
    prow = _perm_rows(src)  # permuted node-table row of the source
    c = np.searchsorted(QGOFF, prow, side="right") - 1  # quarter of the source

    # sort edges by (core, s, c, trel)
    key = ((core * NSUP + s) * NCHUNKS + c) * 4 + trel
    order = np.argsort(key, kind="stable")
    ks = key[order]
    src_s = src[order]
    dst_s = dst[order]

    ngroups = NCORES * NSUP * NCHUNKS * 4
    cnt = np.bincount(ks, minlength=ngroups).reshape(NCORES, NSUP, NCHUNKS, 4)

    # global (same for all cores) block counts per (s, c, trel)
    nb = -(-cnt.max(axis=0) // 128)  # ceil div, [NSUP, NCHUNKS, 4]
    nb[:, 0, :] = np.maximum(nb[:, 0, :], 1)  # ensure every tile gets >=1 block
    nblk = int(nb.sum())
    e_pad = nblk * 128

    # slot offsets in the padded edge array, ordered (s, c, trel)
    goff = np.zeros((NSUP, NCHUNKS, 4), dtype=np.int64)
    flat = (nb * 128).reshape(-1)
    goff.reshape(-1)[1:] = np.cumsum(flat)[:-1]

    # position of each edge within its group
    grp_all = np.bincount(ks, minlength=ngroups)
    gstart = np.zeros(ngroups, dtype=np.int64)
    gstart[1:] = np.cumsum(grp_all)[:-1]
    within = np.arange(len(ks), dtype=np.int64) - gstart[ks]

    sg = ks % (NSUP * NCHUNKS * 4)
    slot = goff.reshape(-1)[sg] + within
    edge_core = ks // (NSUP * NCHUNKS * 4)

    src_rel_pad = np.zeros((NCORES, e_pad), dtype=np.int16)
    dst_loc_pad = np.full((NCORES, e_pad), PAD_DST, dtype=np.float16)
    prow_s = _perm_rows(src_s)
    qs_ = np.searchsorted(QGOFF, prow_s, side="right") - 1
    src_rel_pad[edge_core, slot] = (prow_s - QGOFF[qs_]).astype(np.int16)
    dst_loc_pad[edge_core, slot] = (dst_s % SHARD - (dst_s % SHARD) // 128 * 128).astype(
        np.float16
    )

    # device layouts: indices in 16-partition rows (replicated to 128 on device)
    idx16 = np.ascontiguousarray(
        src_rel_pad.reshape(NCORES, e_pad // 16, 16).transpose(0, 2, 1)
    )  # [NCORES, 16, e_pad//16]
    dstloc = np.ascontiguousarray(
        dst_loc_pad.reshape(NCORES, nblk, 128).transpose(0, 2, 1)
    )  # [NCORES, 128, nblk] fp16

    # per-supertile gather call sizes and per-tile block lists
    nb_sc = nb.sum(axis=2)  # [NSUP, NCHUNKS] blocks per gather call
    blocks_of_tile = []  # global tile 0..99 -> list of G-column indices
    sup_cols = []  # supertile -> (start block col in dstloc, total blocks)
    bbase = 0
    for si in range(NSUP):
        tot = int(nb_sc[si].sum())
        sup_cols.append((bbase, tot))
        off = 0
        per_tile = [[] for _ in range(4)]
        for ci in range(NCHUNKS):
            for tr in range(4):
                for b in range(int(nb[si, ci, tr])):
                    per_tile[tr].append(off)
                    off += 1
        for tr in range(4):
            blocks_of_tile.append(per_tile[tr])
        bbase += tot
    assert bbase == nblk

    return {
        "idx16": idx16,
        "dstloc": dstloc,
        "nblk": nblk,
        "e_pad": e_pad,
        "nb": nb,
        "nb_sc": nb_sc,
        "blocks_of_tile": blocks_of_tile,
        "sup_cols": sup_cols,
    }


def _build_pool_layout(batch):
    batch = np.asarray(batch, dtype=np.int64)
    slot_graphs = []  # per core: list of graph ids
    slot_ranges = []  # per core: list of (start, count) local node ranges
    for m in range(NCORES):
        bm = batch[m * SHARD : (m + 1) * SHARD]
        gs, starts, cnts = np.unique(bm, return_index=True, return_counts=True)
        slot_graphs.append(list(gs))
        slot_ranges.append(list(zip(starts.tolist(), cnts.tolist())))
    nslots = max(len(g) for g in slot_graphs)
    nchk = max(
        -(-cnt // 128) for rs in slot_ranges for (_, cnt) in rs
    )  # chunks of 128 per slot

    pool_ids = np.zeros((NCORES, nslots * nchk * 128), dtype=np.int16)
    for m in range(NCORES):
        for j in range(nslots):
            base = j * nchk * 128
            if j < len(slot_ranges[m]):
                start, cnt = slot_ranges[m][j]
                ids = np.full(nchk * 128, start, dtype=np.int16)
                ids[:cnt] = np.arange(start, start + cnt, dtype=np.int16)
            else:
                ids = np.zeros(nchk * 128, dtype=np.int16)
            pool_ids[m, base : base + nchk * 128] = ids

    npool = nslots * nchk * 128
    pool_idx16 = np.ascontiguousarray(
        pool_ids.reshape(NCORES, npool // 16, 16).transpose(0, 2, 1)
    )  # [NCORES, 16, npool//16]
    return {
        "pool_idx16": pool_idx16,
        "nslots": nslots,
        "nchk": nchk,
        "slot_graphs": slot_graphs,
    }


def _build_nc(lay, pool_lay, sim=False, ablate=(), repeat=1):
    ablate = set(ablate)
    nblk = lay["nblk"]
    e_pad = lay["e_pad"]
    nb = lay["nb"]
    nb_sc = lay["nb_sc"]
    sup_cols = lay["sup_cols"]
    nslots = pool_lay["nslots"]
    nchk = pool_lay["nchk"]

    nc = bacc.Bacc("TRN2", target_bir_lowering=False, debug=False, num_devices=NCORES,
                   num_swdge_queues=NQUEUES)

    xsh_in = nc.dram_tensor("x_sh", [SHARD, D], F16, kind="ExternalInput")
    xfm_in = nc.dram_tensor("x_fm", [D, NTILES * 128], F32, kind="ExternalInput")
    idx_in = nc.dram_tensor("idx16", [16, e_pad // 16], I16, kind="ExternalInput")
    dst_in = nc.dram_tensor("dstloc", [128, nblk], F16, kind="ExternalInput")
    w_in = nc.dram_tensor("w", [L * K * 128, 128], F32, kind="ExternalInput")
    sb_in = nc.dram_tensor("scale_bias", [128, 2 * L * K], F32, kind="ExternalInput")
    iota_in = nc.dram_tensor("iota", [128, GMAX * 128], F16, kind="ExternalInput")
    pidx_in = nc.dram_tensor(
        "pool_idx16", [16, nslots * nchk * 8], I16, kind="ExternalInput"
    )
    pooled_out = nc.dram_tensor("pooled", [128, nslots], F32, kind="ExternalOutput")

    with tile.TileContext(nc) as tc:
        es = ExitStack()
        with es:
            const = es.enter_context(tc.tile_pool(name="const", bufs=1))
            gpool = es.enter_context(tc.tile_pool(name="g", bufs=8))
            ohpool = es.enter_context(tc.tile_pool(name="oh", bufs=4))
            spool = es.enter_context(tc.tile_pool(name="s", bufs=3))
            tnpool = es.enter_context(tc.tile_pool(name="tn", bufs=4))
            stpool = es.enter_context(tc.tile_pool(name="st", bufs=2))
            agg_ps = es.enter_context(tc.tile_pool(name="aggps", bufs=2, space="PSUM"))
            y_ps = es.enter_context(tc.tile_pool(name="yps", bufs=2, space="PSUM"))
            tp_ps = es.enter_context(tc.tile_pool(name="tpps", bufs=2, space="PSUM"))
            dram = es.enter_context(tc.tile_pool(name="dram", bufs=1, space="DRAM"))

            # --- resident SBUF constants (indices replicated 16->128 on device)
            idx_sb = const.tile([128, e_pad // 16], I16)
            for r in range(8):
                nc.sync.dma_start(out=idx_sb[16 * r : 16 * (r + 1), :], in_=idx_in[:, :])
            dst_sb = const.tile([128, nblk], F16)
            nc.sync.dma_start(out=dst_sb[:], in_=dst_in[:, :])
            iota_sb = const.tile([128, GMAX, 128], F16)
            nc.sync.dma_start(out=iota_sb[:], in_=iota_in[:, :])
            sb_sb = const.tile([128, 2 * L * K], F32)
            nc.sync.dma_start(out=sb_sb[:], in_=sb_in[:, :])
            pidx_sb = const.tile([128, nslots * nchk * 8], I16)
            for r in range(8):
                nc.sync.dma_start(
                    out=pidx_sb[16 * r : 16 * (r + 1), :], in_=pidx_in[:, :]
                )
            w_sb = []
            for lk in range(L * K):
                w_t = const.tile([128, 128], F32, tag=f"w{lk}", name=f"w{lk}")
                nc.sync.dma_start(out=w_t[:], in_=w_in[lk * 128 : (lk + 1) * 128, :])
                w_sb.append(w_t)
            ident = const.tile([128, 128], F32)
            make_identity(nc, ident[:])

            # --- internal DRAM; node tables split per quarter so a gather
            # from quarter q only depends on quarter q's AllGather
            x_nm = [
                dram.tile([int(QSIZE[q]) * NCORES, D], F16, tag=f"xnm{q}", name=f"xnm{q}")
                for q in range(4)
            ]
            h_nm = [
                [
                    dram.tile([int(QSIZE[q]) * NCORES, D], F16, tag=f"hnm{i}q{q}",
                              name=f"hnm{i}q{q}")
                    for q in range(4)
                ]
                for i in range(2)
            ]
            h_fm = [dram.tile([D, NTILES * 128], F32, tag=f"hfm{i}", name=f"hfm{i}") for i in range(2)]
            shard_nm = [dram.tile([SHARD, D], F16, tag=f"shard{i}", name=f"shard{i}") for i in range(2)]
            h_pool = dram.tile([SHARD, D], F32, tag="hpool")
            x_stage = dram.tile([SHARD, D], F16, tag="xstage")

            for _rep in range(repeat):
                # stage x: quarter AllGathers of each core's node-major shard
                # (collectives cannot read IO tensors, so bounce via x_stage)
                nc.sync.dma_start(out=x_stage[:, :], in_=xsh_in[:, :])
                for qq in range(4):
                    qs, qsz = int(QSTART[qq]), int(QSIZE[qq])
                    if sim or "stage" in ablate:
                        nc.sync.dma_start(
                            out=x_nm[qq][0:qsz, :], in_=x_stage[qs : qs + qsz, :]
                        )
                    else:
                        nc.gpsimd.collective_compute(
                            "AllGather",
                            mybir.AluOpType.bypass,
                            replica_groups=[list(range(NCORES))],
                            ins=[x_stage[qs : qs + qsz, :].opt()],
                            outs=[x_nm[qq][0 : qsz * NCORES, :].opt()],
                        )

                qrr = [0]
                for l in range(L):
                    tabs = x_nm if l == 0 else h_nm[l - 1]
                    chunk_views = [tabs[ci][:, :] for ci in range(NCHUNKS)]

                    for si in range(NSUP):
                        bbase, btot = sup_cols[si]
                        agg = None
                        if "aggmm" not in ablate:
                            agg = agg_ps.tile([128, 512], F32, space="PSUM", tag="agg")
                        colbase = bbase * 8
                        sup_off = 0
                        for ci in range(NCHUNKS):
                            nbi = int(nb_sc[si, ci])
                            if nbi == 0:
                                continue
                            # per-tile-region block counts for this chunk, in
                            # consumption order
                            blk_tr = []
                            for tr in range(4):
                                blk_tr += [tr] * int(nb[si, ci, tr])
                            # split the chunk's blocks into <=GMAX-block
                            # gathers so several stay in flight across the 4
                            # SWDGE queues
                            for gstart in range(0, nbi, GMAX):
                                gcnt = min(GMAX, nbi - gstart)
                                nidx = gcnt * 128
                                g_t = None
                                if not ("gather" in ablate and "aggmm" in ablate):
                                    g_t = gpool.tile([128, gcnt, 128], F16, tag="g")
                                if "gather" not in ablate:
                                    nc.gpsimd.dma_gather(
                                        out_ap=g_t[:],
                                        in_ap=chunk_views[ci],
                                        idxs_ap=idx_sb[:, colbase : colbase + gcnt * 8],
                                        num_idxs=nidx,
                                        num_idxs_reg=nidx,
                                        elem_size=D,
                                        single_packet=False,
                                        queue_num=qrr[0] % NQUEUES,
                                    )
                                    qrr[0] += 1
                                colbase += gcnt * 8
                                bcol = bbase + sup_off
                                if "onehot" not in ablate:
                                    # one is_equal builds all gcnt one-hots
                                    oh = ohpool.tile([128, gcnt, 128], F16, tag="oh")
                                    nc.vector.tensor_tensor(
                                        out=oh[:],
                                        in0=iota_sb[:, :gcnt, :],
                                        in1=dst_sb[:, bcol : bcol + gcnt].to_broadcast(
                                            [128, gcnt, 128]
                                        ),
                                        op=mybir.AluOpType.is_equal,
                                    )
                                for off in range(gcnt):
                                    tr = blk_tr[gstart + off]
                                    if "onehot" in ablate:
                                        oh_blk = iota_sb[:, off, :]
                                    else:
                                        oh_blk = oh[:, off, :]
                                    # One accumulation group per PSUM bank:
                                    # start=True clears has_written for the WHOLE
                                    # bank, so only the supertile's first matmul
                                    # may set it. Per-element has_written then
                                    # overwrites on each region's first write and
                                    # accumulates afterwards.
                                    if "aggmm" not in ablate:
                                        nc.tensor.matmul(
                                            out=agg[:, tr * 128 : (tr + 1) * 128],
                                            lhsT=g_t[:, off, :],
                                            rhs=oh_blk,
                                            start=(sup_off == 0),
                                            stop=(sup_off == btot - 1),
                                            skip_group_check=True,
                                        )
                                    sup_off += 1

                        # residual + MLP (feature-major [128, 512])
                        hfm_t = spool.tile([128, 512], F32, tag="hfm_t")
                        if l == 0:
                            nc.sync.dma_start(
                                out=hfm_t[:], in_=xfm_in[:, si * 512 : (si + 1) * 512]
                            )
                        else:
                            nc.sync.dma_start(
                                out=hfm_t[:],
                                in_=h_fm[(l - 1) % 2][:, si * 512 : (si + 1) * 512],
                            )
                        u = spool.tile([128, 512], F32, tag="u")
                        if "aggmm" in ablate:
                            nc.vector.tensor_copy(out=u[:], in_=hfm_t[:])
                        else:
                            nc.vector.tensor_tensor(
                                out=u[:], in0=hfm_t[:], in1=agg[:], op=mybir.AluOpType.add
                            )
                        cur = u
                        for k in range(K) if "mlp" not in ablate else []:
                            y = y_ps.tile([128, 512], F32, space="PSUM", tag="y")
                            nc.tensor.matmul(
                                out=y[:], lhsT=w_sb[l * K + k][:], rhs=cur[:],
                                start=True, stop=True,
                            )
                            v = spool.tile([128, 512], F32, tag=f"v{k}")
                            col = 2 * (l * K + k)
                            nc.scalar.activation(
                                out=v[:],
                                in_=y[:],
                                func=mybir.ActivationFunctionType.Relu,
                                scale=sb_sb[:, col : col + 1],
                                bias=sb_sb[:, col + 1 : col + 2],
                            )
                            cur = v

                        if l < L - 1:
                            nc.sync.dma_start(
                                out=h_fm[l % 2][:, si * 512 : (si + 1) * 512], in_=cur[:]
                            )
                        # node-major writeback via PE transpose
                        for q in range(4):
                            if "transpose" in ablate:
                                continue
                            gt = si * 4 + q
                            row0 = gt * 128
                            if row0 >= SHARD:
                                continue
                            rows = min(128, SHARD - row0)
                            tp = tp_ps.tile([128, 128], F32, space="PSUM", tag="tp")
                            nc.tensor.transpose(
                                out=tp[:], in_=cur[:, q * 128 : (q + 1) * 128],
                                identity=ident[:],
                            )
                            tn = tnpool.tile(
                                [128, 128], F16 if l < L - 1 else F32, tag="tn"
                            )
                            nc.vector.tensor_copy(out=tn[:], in_=tp[:])
                            if l < L - 1:
                                nc.sync.dma_start(
                                    out=shard_nm[l][row0 : row0 + rows, :],
                                    in_=tn[:rows, :],
                                )
                            else:
                                nc.sync.dma_start(
                                    out=h_pool[row0 : row0 + rows, :], in_=tn[:rows, :]
                                )

                        if l < L - 1 and si in QSUP_LAST and "transpose" not in ablate:
                            qq = QSUP_LAST.index(si)
                            qs, qsz = int(QSTART[qq]), int(QSIZE[qq])
                            if sim:
                                nc.sync.dma_start(
                                    out=h_nm[l][qq][0:qsz, :],
                                    in_=shard_nm[l][qs : qs + qsz, :],
                                )
                            else:
                                nc.gpsimd.collective_compute(
                                    "AllGather",
                                    mybir.AluOpType.bypass,
                                    replica_groups=[list(range(NCORES))],
                                    ins=[shard_nm[l][qs : qs + qsz, :].opt()],
                                    outs=[h_nm[l][qq][0 : qsz * NCORES, :].opt()],
                                )

                # --- pooling: per-slot gather + transpose + reduce_max
                pooled_sb = const.tile([128, nslots], F32)
                for j in range(nslots):
                    pg = gpool.tile([128, nchk, 128], F32, tag="pg", bufs=2)
                    nc.gpsimd.dma_gather(
                        out_ap=pg[:],
                        in_ap=h_pool[:],
                        idxs_ap=pidx_sb[:, j * nchk * 8 : (j + 1) * nchk * 8],
                        num_idxs=nchk * 128,
                        num_idxs_reg=nchk * 128,
                        elem_size=D,
                        single_packet=False,
                        queue_num=qrr[0] % NQUEUES,
                    )
                    qrr[0] += 1
                    stg = stpool.tile([128, nchk * 128], F32, tag="stg")
                    for b in range(nchk):
                        tp = tp_ps.tile([128, 128], F32, space="PSUM", tag="tp")
                        nc.tensor.transpose(
                            out=tp[:], in_=pg[:, b, :], identity=ident[:]
                        )
                        nc.vector.tensor_copy(
                            out=stg[:, b * 128 : (b + 1) * 128], in_=tp[:]
                        )
                    nc.vector.reduce_max(
                        out=pooled_sb[:, j : j + 1], in_=stg[:], axis=mybir.AxisListType.X
                    )
                nc.sync.dma_start(out=pooled_out[:, :], in_=pooled_sb[:])

    nc.compile()
    return nc


def _make_runner(nc, n_cores=NCORES):
    """Build a jitted shard_map callable for nc once; reused across calls."""
    import jax

    from concourse.bass2jax import (
        _bass_exec_p,
        install_neuronx_cc_hook,
        partition_id_tensor,
    )
    from jax.experimental.shard_map import shard_map
    from jax.sharding import Mesh, PartitionSpec

    install_neuronx_cc_hook()

    partition_name = nc.partition_id_tensor.name if nc.partition_id_tensor else None
    in_names, out_names, out_avals, zero_outs = [], [], [], []
    for alloc in nc.m.functions[0].allocations:
        if not isinstance(alloc, mybir.MemoryLocationSet):
            continue
        name = alloc.memorylocations[0].name
        if alloc.kind == "ExternalInput":
            if name != partition_name:
                in_names.append(name)
        elif alloc.kind == "ExternalOutput":
            out_names.append(name)
            shape = tuple(alloc.tensor_shape)
            dtype = mybir.dt.np(alloc.dtype)
            out_avals.append(jax.core.ShapedArray(shape, dtype))
            zero_outs.append(np.zeros(shape, dtype))
    n_params = len(in_names)
    n_outs = len(out_avals)
    all_in_names = list(in_names) + list(out_names)
    if partition_name is not None:
        all_in_names.append(partition_name)

    def _body(*args):
        operands = list(args)
        if partition_name is not None:
            operands.append(partition_id_tensor())
        outs = _bass_exec_p.bind(
            *operands,
            out_avals=tuple(out_avals),
            in_names=tuple(all_in_names),
            out_names=tuple(out_names),
            lowering_input_output_aliases=(),
            sim_require_finite=True,
            sim_require_nnan=True,
            nc=nc,
        )
        return tuple(outs)

    devices = jax.devices()[:n_cores]
    mesh = Mesh(np.asarray(devices), ("core",))
    in_specs = (PartitionSpec("core"),) * (n_params + n_outs)
    out_specs = (PartitionSpec("core"),) * n_outs
    jitted = jax.jit(
        shard_map(_body, mesh=mesh, in_specs=in_specs, out_specs=out_specs,
                  check_rep=False),
        keep_unused=True,
    )
    return jitted, (in_names, out_names, zero_outs)


def _run(sig, in_maps):
    import jax

    jitted, (in_names, out_names, zero_outs) = _runners[sig]
    concat = [
        np.concatenate([np.asarray(in_maps[c][name]) for c in range(NCORES)], axis=0)
        for name in in_names
    ]
    concat += [np.concatenate([z] * NCORES, axis=0) for z in zero_outs]
    outs = jitted(*concat)
    outs = [np.asarray(o) for o in outs]
    per_core = []
    for c in range(NCORES):
        d = {}
        for name, o in zip(out_names, outs):
            sh0 = o.shape[0] // NCORES
            d[name] = o[c * sh0 : (c + 1) * sh0]
        per_core.append(d)
    return per_core


def _make_in_maps(inputs, lay, pool_lay):
    x = np.asarray(inputs["x"], dtype=np.float32)
    Ws = np.asarray(inputs["Ws"], dtype=np.float32)
    bs = np.asarray(inputs["bs"], dtype=np.float32)
    gammas = np.asarray(inputs["gammas"], dtype=np.float32)
    betas = np.asarray(inputs["betas"], dtype=np.float32)
    run_means = np.asarray(inputs["run_means"], dtype=np.float32)
    run_vars = np.asarray(inputs["run_vars"], dtype=np.float32)

    # host-side folded BN params: relu(y*scale + bias')
    scale = gammas / np.sqrt(run_vars + BN_EPS)  # [L, K, D]
    bias = (bs - run_means) * scale + betas  # [L, K, D]
    sb_arr = np.zeros((128, 2 * L * K), dtype=np.float32)
    w_arr = np.zeros((L * K * 128, 128), dtype=np.float32)
    for l in range(L):
        for k in range(K):
            lk = l * K + k
            sb_arr[:, 2 * lk] = scale[l, k]
            sb_arr[:, 2 * lk + 1] = bias[l, k]
            w_arr[lk * 128 : (lk + 1) * 128, :] = Ws[l, k]

    iota = np.tile(np.arange(128, dtype=np.float16)[None, :], (128, GMAX))
    x16 = x.astype(np.float16)

    in_maps = []
    for m in range(NCORES):
        xsh = x16[m * SHARD : (m + 1) * SHARD]
        xfm = np.zeros((D, NTILES * 128), dtype=np.float32)
        xfm[:, :SHARD] = x[m * SHARD : (m + 1) * SHARD].T
        in_maps.append(
            {
                "x_sh": xsh,
                "x_fm": xfm,
                "idx16": lay["idx16"][m],
                "dstloc": lay["dstloc"][m],
                "w": w_arr,
                "scale_bias": sb_arr,
                "iota": iota,
                "pool_idx16": pool_lay["pool_idx16"][m],
            }
        )
    return in_maps


def _finalize(results, pool_lay, inputs):
    lin_W = np.asarray(inputs["lin_W"], dtype=np.float32)
    lin_b = np.asarray(inputs["lin_b"], dtype=np.float32)
    pooled_full = np.full((G, D), -np.inf, dtype=np.float32)
    for m in range(NCORES):
        pm = results[m]["pooled"]  # [128, nslots]
        for j, g in enumerate(pool_lay["slot_graphs"][m]):
            pooled_full[g] = np.maximum(pooled_full[g], pm[:, j])

    logits = pooled_full @ lin_W + lin_b
    mx = logits.max(axis=-1, keepdims=True)
    z = logits - mx
    out = z - np.log(np.exp(z).sum(axis=-1, keepdims=True))
    return out.astype(np.float32)


def _layouts(edge_index, batch):
    ekey = hashlib.sha1(np.ascontiguousarray(edge_index).tobytes()).hexdigest()
    bkey = hashlib.sha1(np.ascontiguousarray(batch).tobytes()).hexdigest()
    key = (ekey, bkey)
    if key not in _layout_cache:
        _layout_cache[key] = (
            _build_edge_layout(edge_index),
            _build_pool_layout(batch),
        )
    return _layout_cache[key]


def kernel(x, edge_index, batch, Ws, bs, gammas, betas, run_means, run_vars, lin_W, lin_b):
    inputs = dict(x=x, edge_index=edge_index, batch=batch, Ws=Ws, bs=bs,
                  gammas=gammas, betas=betas, run_means=run_means,
                  run_vars=run_vars, lin_W=lin_W, lin_b=lin_b)
    edge_index = np.asarray(edge_index)
    batch = np.asarray(batch)

    lay, pool_lay = _layouts(edge_index, batch)

    sig = (lay["nblk"], pool_lay["nslots"], pool_lay["nchk"])
    if sig not in _compiled:
        _compiled[sig] = _build_nc(lay, pool_lay)
    nc = _compiled[sig]
    if sig not in _runners:
        _runners[sig] = _make_runner(nc)

    in_maps = _make_in_maps(inputs, lay, pool_lay)
    results = _run(sig, in_maps)
    out = _finalize(results, pool_lay, inputs)
    if not np.isfinite(out).all():
        # rare transient produced non-finite pooled values: rerun once
        sys.stderr.write("kernel: non-finite output, retrying once\n")
        results = _run(sig, in_maps)
        out = _finalize(results, pool_lay, inputs)
    return out
